# revision 1
# baseline (speedup 1.0000x reference)
"""Trainium2 Bass kernel for per-token quadratic feature map.

reference: x [B=4, H=16, S=4096, d=16] f32 ->
  out [B, H, S, 1 + d + d*d = 273] = concat([1, x/sqrt(sqrt(d)), (x_i*x_j)/(sqrt(2)*sqrt(d))])

Fully data-parallel per (b, h) slice: 64 slices sharded 8 per NeuronCore
across 8 cores. Per core: 8 tiles of 4096 tokens; each tile is
[128 partitions x 32 tokens], output written as one contiguous
[128, 32*273] f32 DMA (4.47 MB).
"""

import math

import numpy as np

B, H, S, D = 4, 16, 4096, 16
BH = B * H                      # 64 (b,h) slices
N_CORES = 8
SLICES_PER_CORE = BH // N_CORES  # 8
TOK_PER_CORE = SLICES_PER_CORE * S  # 32768
NT = 32                          # tokens per partition per tile
P = 128                          # partitions
TILE_TOK = P * NT                # 4096 tokens = one (b,h) slice
OUT_W = 1 + D + D * D            # 273

R2 = math.sqrt(2.0)
RD = math.sqrt(D)
RRD = math.sqrt(RD)
C_LIN = 1.0 / RRD                # linear-term scale
C_SQ = 1.0 / math.sqrt(R2 * RD)  # prescale: (x_i*C_SQ)*(x_j*C_SQ) = x_i*x_j/(R2*RD)

_CACHE = {}


def _build_program():
    from contextlib import ExitStack

    import concourse.bacc as bacc
    import concourse.mybir as mybir
    import concourse.tile as tile

    nc = bacc.Bacc("TRN2", target_bir_lowering=False, debug=False)
    x_d = nc.dram_tensor("x", [TOK_PER_CORE, D], mybir.dt.float32,
                         kind="ExternalInput")
    o_d = nc.dram_tensor("out", [TOK_PER_CORE, OUT_W], mybir.dt.float32,
                         kind="ExternalOutput")

    # flat views: per tile, both input and output regions are contiguous
    x_flat = x_d.ap().rearrange("t d -> (t d)")
    o_flat = o_d.ap().rearrange("t d -> (t d)")

    n_tiles = TOK_PER_CORE // TILE_TOK  # 8

    with tile.TileContext(nc) as tc, ExitStack() as ctx:
        xp = ctx.enter_context(tc.tile_pool(name="x", bufs=3))
        yp = ctx.enter_context(tc.tile_pool(name="y", bufs=3))
        op = ctx.enter_context(tc.tile_pool(name="o", bufs=3))

        for it in range(n_tiles):
            xt = xp.tile([P, NT * D], mybir.dt.float32)
            yt = yp.tile([P, NT * D], mybir.dt.float32)
            ot = op.tile([P, NT * OUT_W], mybir.dt.float32)

            # load 4096 tokens: partition p holds tokens [p*NT, (p+1)*NT)
            src = x_flat[it * TILE_TOK * D:(it + 1) * TILE_TOK * D]
            nc.sync.dma_start(xt[:], src.rearrange("(p f) -> p f", p=P))

            ot3 = ot[:].rearrange("p (t f) -> p t f", f=OUT_W)

            # ones column (gpsimd so DVE/ACT stay free)
            nc.gpsimd.memset(ot3[:, :, 0:1], 1.0)

            # linear term on ScalarE: out[:, t, 1:17] = x * C_LIN
            x3 = xt[:].rearrange("p (t f) -> p t f", f=D)
            nc.scalar.mul(ot3[:, :, 1:1 + D], x3, C_LIN)

            # prescale on ScalarE: y = x * C_SQ
            nc.scalar.mul(yt[:], xt[:], C_SQ)

            # outer products: one DVE tensor_tensor for the whole tile
            y3 = yt[:].rearrange("p (t f) -> p t f", f=D)
            in0 = y3.unsqueeze(3).broadcast_to((P, NT, D, D))  # y[t,i]
            in1 = y3.unsqueeze(2).broadcast_to((P, NT, D, D))  # y[t,j]
            sq = (ot3[:, :, 1 + D:]
                  .rearrange("p t (i j) -> p t i j", j=D))
            nc.vector.tensor_mul(sq, in0, in1)

            # store: contiguous 4.47 MB
            dst = o_flat[it * TILE_TOK * OUT_W:(it + 1) * TILE_TOK * OUT_W]
            nc.sync.dma_start(dst.rearrange("(p f) -> p f", p=P), ot[:])

    nc.compile()
    return nc


def kernel(x: np.ndarray) -> np.ndarray:
    from concourse.bass_utils import run_bass_kernel_spmd

    x = np.ascontiguousarray(np.asarray(x, dtype=np.float32))
    assert x.shape == (B, H, S, D), x.shape

    if "nc" not in _CACHE:
        _CACHE["nc"] = _build_program()
    nc = _CACHE["nc"]

    xr = x.reshape(BH, S, D)
    in_maps = [
        {"x": xr[c * SLICES_PER_CORE:(c + 1) * SLICES_PER_CORE]
            .reshape(TOK_PER_CORE, D)}
        for c in range(N_CORES)
    ]
    res = run_bass_kernel_spmd(nc, in_maps, core_ids=list(range(N_CORES)))
    out = np.concatenate([r["out"].reshape(SLICES_PER_CORE, S, OUT_W)
                          for r in res.results], axis=0)
    return out.reshape(B, H, S, OUT_W)


# revision 6
# speedup vs baseline: 89934.8322x; 89934.8322x over previous
"""Trainium2 Bass kernel for per-token quadratic feature map.

reference: x [B=4, H=16, S=4096, d=16] f32 ->
  out [B, H, S, 1 + d + d*d = 273] = concat([1, x/sqrt(sqrt(d)), (x_i*x_j)/(sqrt(2)*sqrt(d))])

Fully data-parallel per (b, h) slice: 64 slices sharded 8 per NeuronCore
across 8 cores. Per core: 8 tiles of 4096 tokens; each tile is
[128 partitions x 32 tokens], output written as one contiguous
[128, 32*273] f32 DMA (4.47 MB).
"""

import math

import numpy as np

B, H, S, D = 4, 16, 4096, 16
BH = B * H                      # 64 (b,h) slices
N_CORES = 8
SLICES_PER_CORE = BH // N_CORES  # 8
TOK_PER_CORE = SLICES_PER_CORE * S  # 32768
NT = 32                          # tokens per partition per tile
P = 128                          # partitions
TILE_TOK = P * NT                # 4096 tokens = one (b,h) slice
OUT_W = 1 + D + D * D            # 273

R2 = math.sqrt(2.0)
RD = math.sqrt(D)
RRD = math.sqrt(RD)
C_LIN = 1.0 / RRD                # linear-term scale
C_SQ = 1.0 / math.sqrt(R2 * RD)  # prescale: (x_i*C_SQ)*(x_j*C_SQ) = x_i*x_j/(R2*RD)

_CACHE = {}


def build_program(reps=1, loop_reps=0):
    """Build + compile the per-core Bass program. `reps` statically repeats
    the whole 8-tile pipeline; `loop_reps` wraps it in a hardware For_i loop
    (both used only for HW timing via slope)."""
    from contextlib import ExitStack

    import concourse.bacc as bacc
    import concourse.mybir as mybir
    import concourse.tile as tile

    nc = bacc.Bacc("TRN2", target_bir_lowering=False, debug=False)
    x_d = nc.dram_tensor("x", [TOK_PER_CORE, D], mybir.dt.float32,
                         kind="ExternalInput")
    o_d = nc.dram_tensor("out", [TOK_PER_CORE, OUT_W], mybir.dt.float32,
                         kind="ExternalOutput")

    # flat views: per tile, both input and output regions are contiguous
    x_flat = x_d.ap().rearrange("t d -> (t d)")
    o_flat = o_d.ap().rearrange("t d -> (t d)")

    # Tile-size ladder (tokens per partition per tile): small first tiles so
    # the first out-DMA launches early; 32-token tiles in steady state.
    ladder = [4, 4, 8, 16] + [NT] * 7
    assert sum(ladder) == TOK_PER_CORE // P

    with tile.TileContext(nc) as tc, ExitStack() as ctx:
        xp = ctx.enter_context(tc.tile_pool(name="x", bufs=6))
        yp = ctx.enter_context(tc.tile_pool(name="y", bufs=6))
        op = ctx.enter_context(tc.tile_pool(name="o", bufs=4))
        if loop_reps:
            ctx.enter_context(tc.For_i(0, loop_reps, 1))

        for _ in range(reps):
            pos = 0
            for nt in ladder:
                tile_tok = P * nt
                xt = xp.tile([P, nt * D], mybir.dt.float32, tag="xt")
                yt = yp.tile([P, nt * D], mybir.dt.float32, tag="yt")
                ot = op.tile([P, nt * OUT_W], mybir.dt.float32, tag="ot")

                # load: partition p holds nt consecutive tokens. Issued on
                # the ACT HWDGE ring so loads never queue behind the big
                # out-stores on the SP ring.
                src = x_flat[pos * D:(pos + tile_tok) * D]
                nc.scalar.dma_start(xt[:], src.rearrange("(p f) -> p f", p=P))

                ot3 = ot[:].rearrange("p (t f) -> p t f", f=OUT_W)

                # ones column (gpsimd so DVE/ACT stay free)
                nc.gpsimd.memset(ot3[:, :, 0:1], 1.0)

                # linear term on ScalarE: out[:, t, 1:17] = x * C_LIN
                x3 = xt[:].rearrange("p (t f) -> p t f", f=D)
                nc.scalar.mul(ot3[:, :, 1:1 + D], x3, C_LIN)

                # prescale on ScalarE: y = x * C_SQ
                nc.scalar.mul(yt[:], xt[:], C_SQ)

                # outer products: one DVE tensor_tensor for the whole tile
                y3 = yt[:].rearrange("p (t f) -> p t f", f=D)
                in0 = y3.unsqueeze(3).broadcast_to((P, nt, D, D))  # y[t,i]
                in1 = y3.unsqueeze(2).broadcast_to((P, nt, D, D))  # y[t,j]
                sq = (ot3[:, :, 1 + D:]
                      .rearrange("p t (i j) -> p t i j", j=D))
                nc.vector.tensor_mul(sq, in0, in1)

                # store: contiguous (up to 4.47 MB) on the SP ring
                dst = o_flat[pos * OUT_W:(pos + tile_tok) * OUT_W]
                nc.sync.dma_start(dst.rearrange("(p f) -> p f", p=P), ot[:])
                pos += tile_tok

    nc.compile()
    return nc


def _make_runner(nc):
    """One-time: build a cached jitted shard_map executor for `nc`."""
    import jax
    from jax.experimental.shard_map import shard_map
    from jax.sharding import Mesh, NamedSharding, PartitionSpec

    import concourse.mybir as mybir
    from concourse.bass2jax import (
        _bass_exec_p,
        install_neuronx_cc_hook,
        partition_id_tensor,
    )

    install_neuronx_cc_hook()

    in_names, out_names, out_avals = [], [], []
    pname = nc.partition_id_tensor.name if nc.partition_id_tensor else None
    for alloc in nc.m.functions[0].allocations:
        if not isinstance(alloc, mybir.MemoryLocationSet):
            continue
        name = alloc.memorylocations[0].name
        if alloc.kind == "ExternalInput":
            if name != pname:
                in_names.append(name)
        elif alloc.kind == "ExternalOutput":
            out_names.append(name)
            out_avals.append(jax.core.ShapedArray(
                tuple(alloc.tensor_shape), mybir.dt.np(alloc.dtype)))
    assert in_names == ["x"] and out_names == ["out"], (in_names, out_names)

    all_in = tuple(in_names) + tuple(out_names)
    if pname is not None:
        all_in = all_in + (pname,)
    bind_kwargs = dict(
        out_avals=tuple(out_avals),
        in_names=all_in,
        out_names=tuple(out_names),
        lowering_input_output_aliases=(),
        sim_require_finite=True,
        sim_require_nnan=True,
        nc=nc,
    )

    def _body(x, obuf):
        operands = [x, obuf]
        if pname is not None:
            operands.append(partition_id_tensor())
        (o,) = _bass_exec_p.bind(*operands, **bind_kwargs)
        return (o,)

    mesh = Mesh(np.asarray(jax.devices()[:N_CORES]), ("core",))
    fn = jax.jit(
        shard_map(_body, mesh=mesh,
                  in_specs=(PartitionSpec("core"), PartitionSpec("core")),
                  out_specs=(PartitionSpec("core"),),
                  check_rep=False),
        donate_argnums=(1,),
    )
    sharding = NamedSharding(mesh, PartitionSpec("core"))
    oshape = (N_CORES * out_avals[0].shape[0],) + tuple(out_avals[0].shape[1:])
    odtype = out_avals[0].dtype

    make_zeros = jax.jit(lambda: jax.numpy.zeros(oshape, odtype),
                         out_shardings=sharding)

    def run(x_concat: np.ndarray) -> np.ndarray:
        x_dev = jax.device_put(x_concat, sharding)
        (o,) = fn(x_dev, make_zeros())
        return np.asarray(o)

    return run


def kernel(x: np.ndarray) -> np.ndarray:
    x = np.ascontiguousarray(np.asarray(x, dtype=np.float32))
    assert x.shape == (B, H, S, D), x.shape

    if "run" not in _CACHE:
        _CACHE["nc"] = build_program()
        _CACHE["run"] = _make_runner(_CACHE["nc"])

    # core c gets (b,h) slices [8c, 8c+8) -> concat over cores is just
    # the natural [BH*S, D] layout
    x2 = x.reshape(BH * S, D)
    out = _CACHE["run"](x2)          # [BH*S, OUT_W]
    return out.reshape(B, H, S, OUT_W)
